# revision 1
# baseline (speedup 1.0000x reference)
"""Causal multi-head self-attention (B=8, S=2048, D=384, H=4, Hd=96) on 8
Trainium2 NeuronCores.

Sharding: data-parallel over batch — each core processes one batch element,
weights replicated. No collectives needed.

Per-core algorithm (flash-style, fully SBUF-resident, no attention matrix in
HBM):
  - host passes x[b] pre-transposed as xT [384, 2048] (layout prep only)
  - QT/KT computed per head in [96, S] layout (d on partitions)
  - V' = [V_h | ones] natural layout [S, 97*4] via augmented weight matrix
    (bias + ones column folded into the projection contraction)
  - scoresT[k, q] = KT_h^T @ QT_h per 128-row k-tile and 512-col q-chunk;
    exp on ScalarE (PSUM->SBUF, scale=1/sqrt(Hd) folded in); causal diagonal
    blocks zeroed post-exp by multiplying a 0/1 mask on GpSimd
  - OT' accumulated in PSUM: rows 0..95 = unnormalized head output (d x q),
    row 96 = softmax denominator (from the ones column of V')
  - reciprocal (custom DVE approx) -> partition_broadcast -> multiply
  - output projection per head directly from normalized [96, S] tiles,
    summed in PSUM across heads, bias via a rank-1 ones matmul, DMA to HBM
"""

import os
import sys

sys.path.insert(0, "/opt/trn_rl_repo")

import numpy as np

import concourse.bass as bass
import concourse.tile as tile
from concourse import bacc, mybir
from concourse.bass_utils import run_bass_kernel_spmd

N_CORES = 8
S = 2048
D = 384
H = 4
HD = 96
CH = 512          # q-chunk width (columns per matmul)
NCH = S // CH     # 4 q-chunks
P = 128           # k-tile height / partition dim
KTN = S // P      # 16 k-tiles
SCALE = 1.0 / np.sqrt(HD)

F32 = mybir.dt.float32
MM_DT = os.environ.get("ATTN_MM_DT", "float32r")  # float32r | float32


def _split_groups(n, g=3):
    """Split n k-tiles into exp groups of <=g (PSUM tile = g banks)."""
    out = []
    while n > 0:
        if g == 3 and n == 4:
            out += [2, 2]
            break
        take = min(g, n)
        out.append(take)
        n -= take
    return out


def build_nc(repeat=1, variant=(), loop_n=0):
    nc = bacc.Bacc("TRN2", target_bir_lowering=False, debug=False,
                   enable_asserts=False, num_devices=N_CORES)
    # MF: dtype for tensors feeding matmuls (float32r = single-pass relaxed
    # fp32 on the PE, 4x faster than true fp32; same 4-byte numpy layout)
    MF = mybir.dt.float32r if MM_DT == "float32r" else F32

    xt_d = nc.dram_tensor("xt", [D, S], MF, kind="ExternalInput").ap()
    wq_d = nc.dram_tensor("wq", [D, D], MF, kind="ExternalInput").ap()
    wk_d = nc.dram_tensor("wk", [D, D], MF, kind="ExternalInput").ap()
    wvx_d = nc.dram_tensor("wvx", [D + 1, 97 * H], MF, kind="ExternalInput").ap()
    wo_d = nc.dram_tensor("wo", [D, D], MF, kind="ExternalInput").ap()
    bqh_d = nc.dram_tensor("bqh", [HD, H], F32, kind="ExternalInput").ap()
    bkh_d = nc.dram_tensor("bkh", [HD, H], F32, kind="ExternalInput").ap()
    bo_d = nc.dram_tensor("bo", [1, D], MF, kind="ExternalInput").ap()
    msk_d = nc.dram_tensor("msk", [P, 4 * CH], MF, kind="ExternalInput").ap()
    ones_d = nc.dram_tensor("onesrow", [1, S], MF, kind="ExternalInput").ap()
    out_d = nc.dram_tensor("out", [S, D], F32, kind="ExternalOutput").ap()

    Exp = mybir.ActivationFunctionType.Exp
    mult = mybir.AluOpType.mult

    with tile.TileContext(nc) as tc:
        wpool = tc.alloc_tile_pool(name="w", bufs=1)
        xpool = tc.alloc_tile_pool(name="x", bufs=1)
        qkt_pool = tc.alloc_tile_pool(name="qkt", bufs=1)
        vpool = tc.alloc_tile_pool(name="v", bufs=1)
        ppool = tc.alloc_tile_pool(name="p", bufs=3 if "grp3" in variant else 4)
        onpool = tc.alloc_tile_pool(name="on", bufs=2)
        rpool = tc.alloc_tile_pool(name="r", bufs=3)
        GRP = 3 if "grp3" in variant else 2
        ACC_BUFS = 2 if "grp3" in variant else 4
        qkpool = tc.alloc_tile_pool(name="qkps", bufs=2, space="PSUM")
        accpool = tc.alloc_tile_pool(name="accps", bufs=ACC_BUFS, space="PSUM")

        import contextlib
        loop_ctx = (tc.For_i(0, loop_n, 1) if loop_n
                    else contextlib.nullcontext())
        with loop_ctx:
          for _rep in range(repeat):
              # ---- load weights / constants ----
              xt_sb, wq_sb, wk_sb, wv_sb, wo_sb = [], [], [], [], []
              for t in range(3):
                  xt = xpool.tile([P, S], MF, name=f"xt{t}", tag=f"xt{t}")
                  nc.sync.dma_start(xt[:], xt_d[P * t:P * t + P, :])
                  xt_sb.append(xt)
                  wqt = wpool.tile([P, D], MF, name=f"wq{t}", tag=f"wq{t}")
                  nc.sync.dma_start(wqt[:], wq_d[P * t:P * t + P, :])
                  wq_sb.append(wqt)
                  wkt = wpool.tile([P, D], MF, name=f"wk{t}", tag=f"wk{t}")
                  nc.sync.dma_start(wkt[:], wk_d[P * t:P * t + P, :])
                  wk_sb.append(wkt)
                  wvt = wpool.tile([P, 97 * H], MF, name=f"wv{t}", tag=f"wv{t}")
                  nc.sync.dma_start(wvt[:], wvx_d[P * t:P * t + P, :])
                  wv_sb.append(wvt)
              wvb = wpool.tile([1, 97 * H], MF, name="wvb", tag="wvb")
              nc.sync.dma_start(wvb[:], wvx_d[D:D + 1, :])
              for h in range(H):
                  wot = wpool.tile([HD, D], MF, name=f"wo{h}", tag=f"wo{h}")
                  nc.sync.dma_start(wot[:], wo_d[HD * h:HD * h + HD, :])
                  wo_sb.append(wot)
              bq_sb = wpool.tile([HD, H], F32, name="bq", tag="bq")
              nc.sync.dma_start(bq_sb[:], bqh_d[:, :])
              bk_sb = wpool.tile([HD, H], F32, name="bk", tag="bk")
              nc.sync.dma_start(bk_sb[:], bkh_d[:, :])
              bo_sb = wpool.tile([1, D], MF, name="bo", tag="bo")
              nc.sync.dma_start(bo_sb[:], bo_d[:, :])
              msk_sb = wpool.tile([P, 4 * CH], MF, name="msk", tag="msk")
              nc.sync.dma_start(msk_sb[:], msk_d[:, :])
              ones = wpool.tile([1, S], MF, name="ones", tag="ones")
              nc.sync.dma_start(ones[:], ones_d[:, :])

              # ---- Q/K projections: per-head transposed layout [96, S] ----
              qt_sb, kt_sb = [], []
              for h in range(H):
                  qt = qkt_pool.tile([HD, S], MF, name=f"qt{h}", tag=f"qt{h}")
                  qt_sb.append(qt)
                  kt = qkt_pool.tile([HD, S], MF, name=f"kt{h}", tag=f"kt{h}")
                  kt_sb.append(kt)
              for w_sb, b_sb, dst in ((wq_sb, bq_sb, qt_sb), (wk_sb, bk_sb, kt_sb)):
                  for h in range(H):
                      for ci in range(NCH):
                          ps = accpool.tile([HD, CH], F32, name="projps", tag="acc")
                          for t in range(3):
                              nc.tensor.matmul(
                                  ps[:],
                                  w_sb[t][:, HD * h:HD * h + HD],
                                  xt_sb[t][:, CH * ci:CH * ci + CH],
                                  start=(t == 0), stop=(t == 2))
                          nc.scalar.add(
                              dst[h][:, CH * ci:CH * ci + CH], ps[:], b_sb[:, h:h + 1])

              # ---- V' projection: natural layout [S, 97*H] with ones column ----
              v_sb = []
              for st in range(KTN):
                  ps = accpool.tile([P, 97 * H], F32, name="vps", tag="acc")
                  for t in range(3):
                      nc.tensor.matmul(ps[:], xt_sb[t][:, P * st:P * st + P],
                                       wv_sb[t][:], start=(t == 0), stop=False)
                  nc.tensor.matmul(ps[:], ones[:, 0:P], wvb[:],
                                   start=False, stop=True)
                  vt = vpool.tile([P, 97 * H], MF, name=f"v{st}", tag=f"v{st}")
                  nc.scalar.copy(vt[:], ps[:])
                  v_sb.append(vt)

              # ---- attention ----
              for ci in range(NCH):
                  on_tiles = []
                  for h in range(H):
                      nkt = 4 * (ci + 1)
                      acc = accpool.tile([P, CH], F32, name="acc", tag="acc")
                      kt0 = 0
                      for gsize in _split_groups(nkt, GRP):
                          kts = list(range(kt0, kt0 + gsize))
                          kt0 += gsize
                          qk = qkpool.tile([P, GRP * CH], F32, name="qk", tag="qk")
                          for j, kt in enumerate(kts):
                              nc.tensor.matmul(
                                  qk[:, CH * j:CH * (j + 1)],
                                  kt_sb[h][:, P * kt:P * kt + P],
                                  qt_sb[h][:, CH * ci:CH * ci + CH],
                                  start=True, stop=True)
                          pt = ppool.tile([P, GRP * CH], MF, name="pt", tag="pt")
                          L = CH * gsize
                          nc.scalar.activation(pt[:, :L], qk[:, :L], Exp, scale=float(SCALE))
                          for j, kt in enumerate(kts):
                              rt = P * kt - CH * ci
                              if rt >= 0:
                                  # zero the upper triangle of the 128x128
                                  # diagonal block; cols below rt are skipped
                                  # by the PV matmul
                                  nc.vector.tensor_mul(
                                      pt[:, CH * j + rt:CH * j + rt + P],
                                      pt[:, CH * j + rt:CH * j + rt + P],
                                      msk_sb[:, 0:P])
                          for j, kt in enumerate(kts):
                              rt = P * kt - CH * ci
                              scol = max(rt, 0)
                              nc.tensor.matmul(
                                  acc[0:97, scol:CH],
                                  v_sb[kt][:, 97 * h:97 * h + 97],
                                  pt[:, CH * j + scol:CH * (j + 1)],
                                  start=(kt == 0), stop=(kt == nkt - 1),
                                  skip_group_check=True)
                      # normalize: row 96 of acc is the softmax denominator.
                      # custom-DVE ops and partition_broadcast only address
                      # partition 0, so stage the row there via a small DMA.
                      den96 = rpool.tile([97, CH], F32, name="den96", tag="den96")
                      nc.vector.tensor_copy(den96[96:97, :], acc[96:97, :])
                      den0 = rpool.tile([1, CH], F32, name="den0", tag="den0")
                      nc.sync.dma_start(den0[:], den96[96:97, :])
                      nc.vector.reciprocal_approx_fast(out=den0[:], in_=den0[:])
                      rb = rpool.tile([HD, CH], F32, name="rb", tag="rb")
                      nc.gpsimd.partition_broadcast(rb[:], den0[:], channels=HD)
                      on = onpool.tile([HD, CH], MF, name=f"on{h}", tag=f"on{h}")
                      nc.vector.tensor_tensor(on[:], acc[0:HD, :], rb[:], op=mult)
                      on_tiles.append(on)
                  # output projection for this chunk's 4 row-tiles
                  for sj in range(4):
                      st = 4 * ci + sj
                      fo = accpool.tile([P, D], F32, name="fo", tag="acc")
                      for h in range(H):
                          nc.tensor.matmul(fo[:], on_tiles[h][:, P * sj:P * sj + P],
                                           wo_sb[h][:], start=(h == 0), stop=False)
                      nc.tensor.matmul(fo[:], ones[:, 0:P], bo_sb[:],
                                       start=False, stop=True)
                      fs = onpool.tile([P, D], F32, name="fs", tag="fs", bufs=3)
                      nc.scalar.copy(fs[:], fo[:])
                      nc.sync.dma_start(out_d[P * st:P * st + P, :], fs[:])

        for pool in (accpool, qkpool, rpool, onpool, ppool, vpool,
                     qkt_pool, xpool, wpool):
            pool.release()

    nc.finalize()
    return nc


_NC_CACHE = None


def get_nc():
    global _NC_CACHE
    if _NC_CACHE is None:
        _NC_CACHE = build_nc()
    return _NC_CACHE


def host_prep(x, Wq, bq, Wk, bk, Wv, bv, Wo, bo):
    """Build per-core input maps (layout prep only; all FLOPs run on device)."""
    x = np.ascontiguousarray(np.asarray(x, dtype=np.float32))
    Wq = np.ascontiguousarray(np.asarray(Wq, dtype=np.float32))
    Wk = np.ascontiguousarray(np.asarray(Wk, dtype=np.float32))
    Wv = np.ascontiguousarray(np.asarray(Wv, dtype=np.float32))
    Wo = np.ascontiguousarray(np.asarray(Wo, dtype=np.float32))
    bq = np.asarray(bq, dtype=np.float32)
    bk = np.asarray(bk, dtype=np.float32)
    bv = np.asarray(bv, dtype=np.float32)
    bo = np.asarray(bo, dtype=np.float32)

    wvx = np.zeros((D + 1, 97 * H), np.float32)
    for h in range(H):
        wvx[:D, 97 * h:97 * h + HD] = Wv[:, HD * h:HD * h + HD]
        wvx[D, 97 * h:97 * h + HD] = bv[HD * h:HD * h + HD]
        wvx[D, 97 * h + HD] = 1.0

    jj = np.arange(CH)[None, :]
    pp = np.arange(P)[:, None]
    msk = np.zeros((P, 4 * CH), np.float32)
    for r in range(4):
        msk[:, CH * r:CH * r + CH] = (jj >= P * r + pp).astype(np.float32)

    bqh = np.ascontiguousarray(bq.reshape(H, HD).T)
    bkh = np.ascontiguousarray(bk.reshape(H, HD).T)
    common = dict(wq=Wq, wk=Wk, wvx=wvx, wo=Wo, bqh=bqh, bkh=bkh,
                  bo=np.ascontiguousarray(bo.reshape(1, D)), msk=msk,
                  onesrow=np.ones((1, S), np.float32))
    return [dict(xt=np.ascontiguousarray(x[b].T), **common)
            for b in range(x.shape[0])]


def kernel(**inputs):
    in_maps = host_prep(**inputs)
    nc = get_nc()
    res = run_bass_kernel_spmd(nc, in_maps, core_ids=list(range(N_CORES)))
    return np.stack([res.results[b]["out"] for b in range(N_CORES)], axis=0)

